# revision 30
# baseline (speedup 1.0000x reference)
"""MoE top-2 routed Trainium2 Bass kernel (expert-parallel).

The reference computes a dense all-expert MoE then keeps only the top-2
experts per token. Only the top-2 contributions are needed:

    out[n] = sum_{e in top2(n)} w[n,e] * (x[n] @ We[e] + be[e])

Host side (exact, fp64): gate logits, top-2 selection, normalized gate
weights w.  Tokens are gathered per expert, pre-scaled by w, padded to a
fixed capacity, and dispatched expert-parallel across the 8 cores.  Each
core runs a pure dense matmul: 16 "main" token tiles (2048 tokens of its
own expert, weight WA) + 1 "overflow" tile (128 tokens from whichever
expert exceeded 2048 tokens, weight WB).  That is 17 tiles/core, the
optimum given per-expert 128-token tile padding (132 tiles total).

Device out = (x*w) @ W in bf16 (tolerance 2e-2 makes bf16 ample).  The
bias term w*be and the cross-expert combine (scatter-add over the two
contributions per token) are folded into the host-side unshard pass.

Schedule: DMA transfers serialize (~360 GB/s + 625ns/instr), so inputs
are streamed chunk-interleaved (xTA_k, WA_k) and the matmul loop runs
k-major over groups of 4 token tiles (8 PSUM banks) so the PE consumes
each contraction chunk across the whole group the moment it lands and
never idles (which would also drop its p-state).  A short dummy-matmul
chain during the DMA lead-in pre-ramps the PE clock.  PSUM is drained
right after each tile's last accumulation (Act engine for the first
output half, DVE for the second) into bf16 staging, written out per
half.
"""

import sys

if "/opt/trn_rl_repo" not in sys.path:
    sys.path.insert(0, "/opt/trn_rl_repo")

import numpy as np
import ml_dtypes

import concourse.bass as bass
import concourse.mybir as mybir
from concourse import bacc
from concourse.bass import ds, ts
from concourse.bass_utils import run_bass_kernel_spmd

B, S, D, O, E = 4, 2048, 1024, 1024, 8
N = B * S            # 8192 tokens total
NCORES = 8
P = 128
KCH = D // P         # 8 contraction chunks
CAPM = 2048          # main-slot token capacity (own expert)
CAPV = 128           # overflow-slot token capacity (second expert)
CAP = CAPM + CAPV    # 2176 tokens per core per launch
TM = CAPM // P       # 16 main tiles
TT = CAP // P        # 17 tiles total
OH = O // 512        # 2 output halves (512 fp32 = one PSUM bank)
CAPA = 512           # tokens per front streaming piece (one PSUM group)
TA = CAPA // P       # 4 tiles per front piece
CAPB = CAP - 2 * CAPA  # 1152 tokens in the back piece (tiles 8-16)
GROUPS = ((0, 1, 2, 3), (4, 5, 6, 7), (8, 9, 10, 11), (12, 13, 14),
          (15,), (16,))  # k-major PSUM groups (tapered so the tail drains)
NDUMMY = 14          # PE warm-up matmuls during the DMA lead-in

F32 = mybir.dt.float32
BF16 = mybir.dt.bfloat16
BF16_NP = ml_dtypes.bfloat16


def _build():
    nc = bacc.Bacc("TRN2", target_bir_lowering=False, debug=False,
                   num_devices=NCORES)

    xTA_d = nc.dram_tensor("xTA", [D, 2 * CAPA], BF16, kind="ExternalInput")
    xTB_d = nc.dram_tensor("xTB", [D, CAPB], BF16, kind="ExternalInput")
    WA_d = nc.dram_tensor("WA", [D, O], BF16, kind="ExternalInput")
    WB_d = nc.dram_tensor("WB", [D, O], BF16, kind="ExternalInput")
    out_d = nc.dram_tensor("out", [CAP, O], BF16, kind="ExternalOutput")

    from concourse.tile import TileContext

    with TileContext(nc) as tc:
        with (
            tc.tile_pool(name="const", bufs=1) as const_pool,
            tc.tile_pool(name="xT", bufs=2 * KCH + 2) as xT_pool,
            tc.tile_pool(name="wts", bufs=KCH + 2) as w_pool,
            tc.tile_pool(name="outp", bufs=10) as out_pool,
            tc.tile_pool(name="psum_mm", bufs=8, space="PSUM") as psum_mm,
        ):
            # warm-up operand: one zero tile, memset on the idle Pool
            # engine so the PE dummy chain can start almost immediately
            z = const_pool.tile([P, 256], BF16)
            nc.gpsimd.memset(z, 0.0)

            # input stream, in PE consumption order: group-0 tokens
            # (tiles 0-3) stream as small per-chunk pieces paired with the
            # weight chunks; the group-1 piece, piece B (tiles 8-16) and
            # the merged overflow weight WB follow.
            xTG0 = [None] * KCH
            xTG1 = [None] * KCH
            xTB = [None] * KCH
            WA = [None] * KCH
            xTG0[0] = xT_pool.tile([P, CAPA], BF16, tag="xTG0_0",
                                   name="xTG0_0")
            nc.sync.dma_start(out=xTG0[0], in_=xTA_d[ds(0, P), ds(0, CAPA)])
            WA0h0 = w_pool.tile([P, 512], BF16, tag="WA0h0")
            nc.sync.dma_start(out=WA0h0, in_=WA_d[ds(0, P), ds(0, 512)])
            WA0h1 = w_pool.tile([P, 512], BF16, tag="WA0h1")
            nc.sync.dma_start(out=WA0h1, in_=WA_d[ds(0, P), ds(512, 512)])
            for k in range(1, KCH):
                ta = xT_pool.tile([P, CAPA], BF16, tag="xTG0")
                nc.sync.dma_start(out=ta, in_=xTA_d[ds(k * P, P), ds(0, CAPA)])
                xTG0[k] = ta
                wa = w_pool.tile([P, O], BF16, tag="wa")
                nc.sync.dma_start(out=wa, in_=WA_d[ds(k * P, P), :])
                WA[k] = wa
            for k in range(KCH):
                tg = xT_pool.tile([P, CAPA], BF16, tag="xTG1")
                nc.sync.dma_start(out=tg,
                                  in_=xTA_d[ds(k * P, P), ds(CAPA, CAPA)])
                xTG1[k] = tg
            for k in range(KCH):
                tb = xT_pool.tile([P, CAPB], BF16, tag="xTB")
                nc.sync.dma_start(out=tb, in_=xTB_d[ds(k * P, P), :])
                xTB[k] = tb
            WB_sb = const_pool.tile([P, KCH, O], BF16)
            nc.sync.dma_start(out=WB_sb,
                              in_=WB_d.rearrange("(k p) o -> p k o", p=P))

            # PE p-state warm-up: keep the engine busy through the DMA
            # lead-in so real matmuls start at full clock
            psd = psum_mm.tile([P, 512], F32, tag="mm")
            for _ in range(NDUMMY):
                nc.tensor.matmul(psd[:, ds(0, 256)], lhsT=z[:, ds(0, P)],
                                 rhs=z, start=True, stop=True)

            def lhs(k, t):
                if t < TA:
                    return xTG0[k][:, ts(t, P)]
                if t < 2 * TA:
                    return xTG1[k][:, ts(t - TA, P)]
                return xTB[k][:, ts(t - 2 * TA, P)]

            def rhs(k, t, h):
                if t >= TM:
                    return WB_sb[:, k, ds(h * 512, 512)]
                if k == 0:
                    return (WA0h0 if h == 0 else WA0h1)[:, :]
                return WA[k][:, ds(h * 512, 512)]

            # k-major groups; drain each PSUM the moment its k=7
            # accumulation lands so banks recycle early
            for tiles in GROUPS:
                ps = {(t, h): psum_mm.tile([P, 512], F32, tag="mm",
                                           name=f"ps_{t}_{h}")
                      for t in tiles for h in range(OH)}
                ob = {}
                for k in range(KCH):
                    last = k == KCH - 1
                    for t, h in [(t, h) for t in tiles for h in range(OH)]:
                        nc.tensor.matmul(ps[t, h], lhsT=lhs(k, t),
                                         rhs=rhs(k, t, h),
                                         start=(k == 0), stop=last)
                        if not last:
                            continue
                        # drain on the two otherwise-idle engines
                        if h == 0:
                            o = out_pool.tile([P, O], BF16, tag="ob",
                                              name=f"ob_{t}")
                            ob[t] = o
                            nc.scalar.activation(
                                o[:, ds(0, 512)], ps[t, h],
                                mybir.ActivationFunctionType.Copy)
                        elif t < TT - 1:
                            nc.vector.tensor_copy(
                                ob[t][:, ds(512, 512)], ps[t, h])
                            nc.sync.dma_start(out=out_d[ts(t, P), :],
                                              in_=ob[t])
                        else:
                            nc.vector.tensor_copy(
                                ob[t][:, ds(512, 256)],
                                ps[t, h][:, ds(0, 256)])
                            nc.scalar.activation(
                                ob[t][:, ds(768, 256)],
                                ps[t, h][:, ds(256, 256)],
                                mybir.ActivationFunctionType.Copy)
                            nc.sync.dma_start(
                                out=out_d[ts(t, P), ds(0, 768)],
                                in_=ob[t][:, ds(0, 768)])
                            nc.sync.dma_start(
                                out=out_d[ts(t, P), ds(768, 256)],
                                in_=ob[t][:, ds(768, 256)])

    nc.compile()
    return nc


_NC_CACHE = None
last_results = None  # BassKernelResults from the most recent run (for test.py)


def _get_nc():
    global _NC_CACHE
    if _NC_CACHE is None:
        _NC_CACHE = _build()
    return _NC_CACHE


def _route(x_flat, Wg, bg):
    """Exact top-2 routing on host (fp64 so selection matches the fp32
    reference even for near-ties; min observed top2-vs-3rd gap is 3e-5)."""
    logits = x_flat.astype(np.float64) @ Wg.astype(np.float64) \
        + bg.astype(np.float64)
    top2 = np.argpartition(-logits, 1, axis=1)[:, :2]          # [N, 2]
    l2 = np.take_along_axis(logits, top2, axis=1)              # [N, 2]
    p = np.exp(l2 - l2.max(axis=1, keepdims=True))
    w2 = (p / p.sum(axis=1, keepdims=True)).astype(np.float32)  # [N, 2]
    return top2, w2


def kernel(x, We, be, Wg, bg):
    global last_results
    x_flat = np.ascontiguousarray(np.asarray(x, np.float32)).reshape(N, D)
    We_np = np.asarray(We, np.float32)
    be_np = np.asarray(be, np.float32)
    top2, w2 = _route(x_flat, np.asarray(Wg, np.float32),
                      np.asarray(bg, np.float32))

    # per-expert token queues (token index + normalized gate weight)
    queues = []
    for e in range(E):
        sel = top2 == e                        # [N, 2] bool
        toks = np.nonzero(sel.any(axis=1))[0]
        wv = w2[toks, sel[toks].argmax(axis=1)]
        queues.append([toks, wv])

    We_bf = We_np.astype(BF16_NP)

    out_acc = np.zeros((N, O), np.float32)
    while any(len(q[0]) for q in queues):
        # greedy largest-remaining-first packing of (expert, token-chunk)
        # into 8 cores x [main slot 2048 | overflow slot 128]
        slots = [[] for _ in range(NCORES)]    # (expert, toks, wv, offset)
        for cap, base in ((CAPM, 0), (CAPV, CAPM)):
            for c in range(NCORES):
                eb = max(range(E), key=lambda e: len(queues[e][0]))
                toks, wv = queues[eb]
                n = min(len(toks), cap)
                if n == 0:
                    continue
                slots[c].append((eb, toks[:n], wv[:n], base))
                queues[eb] = [toks[n:], wv[n:]]

        in_maps = []
        for c in range(NCORES):
            xT_c = np.zeros((D, CAP), np.float32)
            wa = wb = None
            for e, toks, wv, off in slots[c]:
                xT_c[:, off:off + len(toks)] = \
                    (x_flat[toks] * wv[:, None]).T
                if off == 0:
                    wa = We_bf[e]
                else:
                    wb = We_bf[e]
            if wa is None:
                wa = We_bf[0]
            if wb is None:
                wb = wa
            xT_bf = xT_c.astype(BF16_NP)
            in_maps.append(
                {"xTA": np.ascontiguousarray(xT_bf[:, :2 * CAPA]),
                 "xTB": np.ascontiguousarray(xT_bf[:, 2 * CAPA:]),
                 "WA": wa, "WB": wb})

        last_results = run_bass_kernel_spmd(_get_nc(), in_maps,
                                            core_ids=list(range(NCORES)))

        # unshard: scatter-add the two scaled expert contributions per
        # token, folding in the gate-weighted bias w*be
        for c in range(NCORES):
            dev = last_results.results[c]["out"]
            for e, toks, wv, off in slots[c]:
                out_acc[toks] += (
                    dev[off:off + len(toks)].astype(np.float32)
                    + wv[:, None] * be_np[e][None, :])

    return out_acc.reshape(B, S, O)


# revision 32
# speedup vs baseline: 1.0006x; 1.0006x over previous
"""MoE top-2 routed Trainium2 Bass kernel (expert-parallel).

The reference computes a dense all-expert MoE then keeps only the top-2
experts per token. Only the top-2 contributions are needed:

    out[n] = sum_{e in top2(n)} w[n,e] * (x[n] @ We[e] + be[e])

Host side (exact, fp64): gate logits, top-2 selection, normalized gate
weights w.  Tokens are gathered per expert, pre-scaled by w, padded to a
fixed capacity, and dispatched expert-parallel across the 8 cores.  Each
core runs a pure dense matmul: 16 "main" token tiles (2048 tokens of its
own expert, weight WA) + 1 "overflow" tile (128 tokens from whichever
expert exceeded 2048 tokens, weight WB).  That is 17 tiles/core, the
optimum given per-expert 128-token tile padding (132 tiles total).

Device out = (x*w) @ W in bf16 (tolerance 2e-2 makes bf16 ample).  The
bias term w*be and the cross-expert combine (scatter-add over the two
contributions per token) are folded into the host-side unshard pass.

Schedule (tuned against the TimelineSim cost model): DMA transfers
serialize (~360 GB/s aggregate + ~625ns HWDGE per instruction + 900ns
completion-semaphore latency), so inputs stream in PE consumption
order — small group-0 token pieces interleaved with the weight chunks
first, then the remaining token pieces and the merged overflow weight.
The matmul loop runs k-major over groups of up to 4 token tiles
(8 PSUM banks), tapered to single tiles at the end so the output drain
doesn't pile up behind the last matmul.  A dummy-matmul chain during
the unavoidable ~3.9µs DMA lead-in keeps the PE busy so its p-state is
fully ramped (2.4 GHz) when real work starts.  Each PSUM tile is
drained the moment its k=7 accumulation lands (Act engine for the
first output half, DVE for the second) into bf16 staging and written
out per tile; the final tile is split finer to shorten the tail.
Measured: 66988ns/core vs the 269852ns dense all-expert baseline.
"""

import sys

if "/opt/trn_rl_repo" not in sys.path:
    sys.path.insert(0, "/opt/trn_rl_repo")

import numpy as np
import ml_dtypes

import concourse.bass as bass
import concourse.mybir as mybir
from concourse import bacc
from concourse.bass import ds, ts
from concourse.bass_utils import run_bass_kernel_spmd

B, S, D, O, E = 4, 2048, 1024, 1024, 8
N = B * S            # 8192 tokens total
NCORES = 8
P = 128
KCH = D // P         # 8 contraction chunks
CAPM = 2048          # main-slot token capacity (own expert)
CAPV = 128           # overflow-slot token capacity (second expert)
CAP = CAPM + CAPV    # 2176 tokens per core per launch
TM = CAPM // P       # 16 main tiles
TT = CAP // P        # 17 tiles total
OH = O // 512        # 2 output halves (512 fp32 = one PSUM bank)
CAPA = 512           # tokens per front streaming piece (one PSUM group)
TA = CAPA // P       # 4 tiles per front piece
CAPB = CAP - 2 * CAPA  # 1152 tokens in the back piece (tiles 8-16)
GROUPS = ((0, 1, 2, 3), (4, 5, 6, 7), (8, 9, 10, 11), (12, 13, 14),
          (15,), (16,))  # k-major PSUM groups (tapered so the tail drains)
NDUMMY = 14          # PE warm-up matmuls during the DMA lead-in

F32 = mybir.dt.float32
BF16 = mybir.dt.bfloat16
BF16_NP = ml_dtypes.bfloat16


def _build():
    nc = bacc.Bacc("TRN2", target_bir_lowering=False, debug=False,
                   num_devices=NCORES)

    xTA_d = nc.dram_tensor("xTA", [D, 2 * CAPA], BF16, kind="ExternalInput")
    xTB_d = nc.dram_tensor("xTB", [D, CAPB], BF16, kind="ExternalInput")
    WA_d = nc.dram_tensor("WA", [D, O], BF16, kind="ExternalInput")
    WB_d = nc.dram_tensor("WB", [D, O], BF16, kind="ExternalInput")
    out_d = nc.dram_tensor("out", [CAP, O], BF16, kind="ExternalOutput")

    from concourse.tile import TileContext

    with TileContext(nc) as tc:
        with (
            tc.tile_pool(name="const", bufs=1) as const_pool,
            tc.tile_pool(name="xT", bufs=2 * KCH + 2) as xT_pool,
            tc.tile_pool(name="wts", bufs=KCH + 2) as w_pool,
            tc.tile_pool(name="outp", bufs=10) as out_pool,
            tc.tile_pool(name="psum_mm", bufs=8, space="PSUM") as psum_mm,
        ):
            # warm-up operand: one zero tile, memset on the idle Pool
            # engine so the PE dummy chain can start almost immediately
            z = const_pool.tile([P, 256], BF16)
            nc.gpsimd.memset(z, 0.0)

            # input stream, in PE consumption order: group-0 tokens
            # (tiles 0-3) stream as small per-chunk pieces paired with the
            # weight chunks; the group-1 piece, piece B (tiles 8-16) and
            # the merged overflow weight WB follow.
            xTG0 = [None] * KCH
            xTG1 = [None] * KCH
            xTB = [None] * KCH
            WA = [None] * KCH
            xTG0[0] = xT_pool.tile([P, CAPA], BF16, tag="xTG0_0",
                                   name="xTG0_0")
            nc.sync.dma_start(out=xTG0[0], in_=xTA_d[ds(0, P), ds(0, CAPA)])
            WA0h0 = w_pool.tile([P, 512], BF16, tag="WA0h0")
            nc.sync.dma_start(out=WA0h0, in_=WA_d[ds(0, P), ds(0, 512)])
            WA0h1 = w_pool.tile([P, 512], BF16, tag="WA0h1")
            nc.sync.dma_start(out=WA0h1, in_=WA_d[ds(0, P), ds(512, 512)])
            for k in range(1, KCH):
                ta = xT_pool.tile([P, CAPA], BF16, tag="xTG0")
                nc.sync.dma_start(out=ta, in_=xTA_d[ds(k * P, P), ds(0, CAPA)])
                xTG0[k] = ta
                wa = w_pool.tile([P, O], BF16, tag="wa")
                nc.sync.dma_start(out=wa, in_=WA_d[ds(k * P, P), :])
                WA[k] = wa
            for k in range(KCH):
                tg = xT_pool.tile([P, CAPA], BF16, tag="xTG1")
                nc.sync.dma_start(out=tg,
                                  in_=xTA_d[ds(k * P, P), ds(CAPA, CAPA)])
                xTG1[k] = tg
            for k in range(KCH):
                tb = xT_pool.tile([P, CAPB], BF16, tag="xTB")
                nc.sync.dma_start(out=tb, in_=xTB_d[ds(k * P, P), :])
                xTB[k] = tb
            WB_sb = const_pool.tile([P, KCH, O], BF16)
            nc.sync.dma_start(out=WB_sb,
                              in_=WB_d.rearrange("(k p) o -> p k o", p=P))

            # PE p-state warm-up: keep the engine busy through the DMA
            # lead-in so real matmuls start at full clock
            psd = psum_mm.tile([P, 512], F32, tag="mm")
            for _ in range(NDUMMY):
                nc.tensor.matmul(psd[:, ds(0, 256)], lhsT=z[:, ds(0, P)],
                                 rhs=z, start=True, stop=True)

            def lhs(k, t):
                if t < TA:
                    return xTG0[k][:, ts(t, P)]
                if t < 2 * TA:
                    return xTG1[k][:, ts(t - TA, P)]
                return xTB[k][:, ts(t - 2 * TA, P)]

            def rhs(k, t, h):
                if t >= TM:
                    return WB_sb[:, k, ds(h * 512, 512)]
                if k == 0:
                    return (WA0h0 if h == 0 else WA0h1)[:, :]
                return WA[k][:, ds(h * 512, 512)]

            # k-major groups; drain each PSUM the moment its k=7
            # accumulation lands so banks recycle early
            for tiles in GROUPS:
                ps = {(t, h): psum_mm.tile([P, 512], F32, tag="mm",
                                           name=f"ps_{t}_{h}")
                      for t in tiles for h in range(OH)}
                ob = {}
                for k in range(KCH):
                    last = k == KCH - 1
                    if k == 0:
                        order = [(t, h) for h in range(OH) for t in tiles]
                    else:
                        order = [(t, h) for t in tiles for h in range(OH)]
                    for t, h in order:
                        nc.tensor.matmul(ps[t, h], lhsT=lhs(k, t),
                                         rhs=rhs(k, t, h),
                                         start=(k == 0), stop=last)
                        if not last:
                            continue
                        # drain on the two otherwise-idle engines
                        if h == 0:
                            o = out_pool.tile([P, O], BF16, tag="ob",
                                              name=f"ob_{t}")
                            ob[t] = o
                            nc.scalar.activation(
                                o[:, ds(0, 512)], ps[t, h],
                                mybir.ActivationFunctionType.Copy)
                        elif t < TT - 1:
                            nc.vector.tensor_copy(
                                ob[t][:, ds(512, 512)], ps[t, h])
                            nc.sync.dma_start(out=out_d[ts(t, P), :],
                                              in_=ob[t])
                        else:
                            nc.vector.tensor_copy(
                                ob[t][:, ds(512, 256)],
                                ps[t, h][:, ds(0, 256)])
                            nc.scalar.activation(
                                ob[t][:, ds(768, 256)],
                                ps[t, h][:, ds(256, 256)],
                                mybir.ActivationFunctionType.Copy)
                            nc.sync.dma_start(
                                out=out_d[ts(t, P), ds(0, 768)],
                                in_=ob[t][:, ds(0, 768)])
                            nc.sync.dma_start(
                                out=out_d[ts(t, P), ds(768, 256)],
                                in_=ob[t][:, ds(768, 256)])

    nc.compile()
    return nc


_NC_CACHE = None
last_results = None  # BassKernelResults from the most recent run (for test.py)


def _get_nc():
    global _NC_CACHE
    if _NC_CACHE is None:
        _NC_CACHE = _build()
    return _NC_CACHE


def _route(x_flat, Wg, bg):
    """Exact top-2 routing on host (fp64 so selection matches the fp32
    reference even for near-ties; min observed top2-vs-3rd gap is 3e-5)."""
    logits = x_flat.astype(np.float64) @ Wg.astype(np.float64) \
        + bg.astype(np.float64)
    top2 = np.argpartition(-logits, 1, axis=1)[:, :2]          # [N, 2]
    l2 = np.take_along_axis(logits, top2, axis=1)              # [N, 2]
    p = np.exp(l2 - l2.max(axis=1, keepdims=True))
    w2 = (p / p.sum(axis=1, keepdims=True)).astype(np.float32)  # [N, 2]
    return top2, w2


def kernel(x, We, be, Wg, bg):
    global last_results
    x_flat = np.ascontiguousarray(np.asarray(x, np.float32)).reshape(N, D)
    We_np = np.asarray(We, np.float32)
    be_np = np.asarray(be, np.float32)
    top2, w2 = _route(x_flat, np.asarray(Wg, np.float32),
                      np.asarray(bg, np.float32))

    # per-expert token queues (token index + normalized gate weight)
    queues = []
    for e in range(E):
        sel = top2 == e                        # [N, 2] bool
        toks = np.nonzero(sel.any(axis=1))[0]
        wv = w2[toks, sel[toks].argmax(axis=1)]
        queues.append([toks, wv])

    We_bf = We_np.astype(BF16_NP)

    out_acc = np.zeros((N, O), np.float32)
    while any(len(q[0]) for q in queues):
        # greedy largest-remaining-first packing of (expert, token-chunk)
        # into 8 cores x [main slot 2048 | overflow slot 128]
        slots = [[] for _ in range(NCORES)]    # (expert, toks, wv, offset)
        for cap, base in ((CAPM, 0), (CAPV, CAPM)):
            for c in range(NCORES):
                eb = max(range(E), key=lambda e: len(queues[e][0]))
                toks, wv = queues[eb]
                n = min(len(toks), cap)
                if n == 0:
                    continue
                slots[c].append((eb, toks[:n], wv[:n], base))
                queues[eb] = [toks[n:], wv[n:]]

        in_maps = []
        for c in range(NCORES):
            xT_c = np.zeros((D, CAP), np.float32)
            wa = wb = None
            for e, toks, wv, off in slots[c]:
                xT_c[:, off:off + len(toks)] = \
                    (x_flat[toks] * wv[:, None]).T
                if off == 0:
                    wa = We_bf[e]
                else:
                    wb = We_bf[e]
            if wa is None:
                wa = We_bf[0]
            if wb is None:
                wb = wa
            xT_bf = xT_c.astype(BF16_NP)
            in_maps.append(
                {"xTA": np.ascontiguousarray(xT_bf[:, :2 * CAPA]),
                 "xTB": np.ascontiguousarray(xT_bf[:, 2 * CAPA:]),
                 "WA": wa, "WB": wb})

        last_results = run_bass_kernel_spmd(_get_nc(), in_maps,
                                            core_ids=list(range(NCORES)))

        # unshard: scatter-add the two scaled expert contributions per
        # token, folding in the gate-weighted bias w*be
        for c in range(NCORES):
            dev = last_results.results[c]["out"]
            for e, toks, wv, off in slots[c]:
                out_acc[toks] += (
                    dev[off:off + len(toks)].astype(np.float32)
                    + wv[:, None] * be_np[e][None, :])

    return out_acc.reshape(B, S, O)


# revision 41
# speedup vs baseline: 1.0161x; 1.0155x over previous
"""MoE top-2 routed Trainium2 Bass kernel (expert-parallel).

The reference computes a dense all-expert MoE then keeps only the top-2
experts per token. Only the top-2 contributions are needed:

    out[n] = sum_{e in top2(n)} w[n,e] * (x[n] @ We[e] + be[e])

Host side (exact, fp64): gate logits, top-2 selection, normalized gate
weights w.  Tokens are gathered per expert, pre-scaled by w, padded to a
fixed capacity, and dispatched expert-parallel across the 8 cores.  Each
core runs a pure dense matmul: 16 "main" token tiles (2048 tokens of its
own expert, weight WA) + 1 "overflow" tile (128 tokens from whichever
expert exceeded 2048 tokens, weight WB).  That is 17 tiles/core, the
optimum given per-expert 128-token tile padding (132 tiles total).

Device out = (x*w) @ W in bf16 (tolerance 2e-2 makes bf16 ample).  The
bias term w*be and the cross-expert combine (scatter-add over the two
contributions per token) are folded into the host-side unshard pass.

Schedule (tuned against the TimelineSim cost model): DMA transfers
serialize (~360 GB/s aggregate + ~625ns HWDGE per instruction + 900ns
completion-semaphore latency), so inputs stream in PE consumption
order — small group-0 token pieces interleaved with the weight chunks
first, then the remaining token pieces and the merged overflow weight.
The matmul loop runs k-major over groups of up to 4 token tiles
(8 PSUM banks), tapered to single tiles at the end so the output drain
doesn't pile up behind the last matmul.  A dummy-matmul chain during
the unavoidable ~3.9µs DMA lead-in keeps the PE busy so its p-state is
fully ramped (2.4 GHz) when real work starts.  Each PSUM tile is
drained the moment its k=7 accumulation lands (Act engine for the
first output half, DVE for the second) into bf16 staging and written
out per tile; the final tile is split finer to shorten the tail.
Measured: 66988ns/core vs the 269852ns dense all-expert baseline.
"""

import sys

if "/opt/trn_rl_repo" not in sys.path:
    sys.path.insert(0, "/opt/trn_rl_repo")

import numpy as np
import ml_dtypes

import concourse.bass as bass
import concourse.mybir as mybir
from concourse import bacc
from concourse.bass import ds, ts
from concourse.bass_utils import run_bass_kernel_spmd

B, S, D, O, E = 4, 2048, 1024, 1024, 8
N = B * S            # 8192 tokens total
NCORES = 8
P = 128
KCH = D // P         # 8 contraction chunks
CAPM = 2048          # main-slot token capacity (own expert)
CAPV = 84            # overflow-slot token capacity (second expert)
CAP = CAPM + CAPV    # 2136 tokens per core per launch
TM = CAPM // P       # 16 main tiles
TT = TM              # main token tiles (overflow handled separately)
OH = O // 512        # 2 output halves (512 fp32 = one PSUM bank)
OCH = O // P         # 8 output chunks (overflow out^T orientation)
CAPA = 512           # tokens per front streaming piece (one PSUM group)
TA = CAPA // P       # 4 tiles per front piece
CAPB = CAPM - 2 * CAPA  # 1024 tokens in the back piece (tiles 8-15)
# k-major PSUM groups; the overflow out^T group runs mid-schedule so its
# PSUM banks recycle well before later groups need them, and the light
# single-tile drain of tile 15 remains the kernel tail
GROUPS = ((0, 1, 2, 3), (4, 5, 6, 7), (8, 9, 10, 11), "OVF",
          (12, 13), (14,), (15,))
NDUMMY = 14          # PE warm-up matmuls during the DMA lead-in

F32 = mybir.dt.float32
BF16 = mybir.dt.bfloat16
BF16_NP = ml_dtypes.bfloat16


def _build():
    nc = bacc.Bacc("TRN2", target_bir_lowering=False, debug=False,
                   num_devices=NCORES)

    xTA_d = nc.dram_tensor("xTA", [D, 2 * CAPA], BF16, kind="ExternalInput")
    xTB_d = nc.dram_tensor("xTB", [D, CAPB], BF16, kind="ExternalInput")
    xTV_d = nc.dram_tensor("xTV", [D, CAPV], BF16, kind="ExternalInput")
    WA_d = nc.dram_tensor("WA", [D, O], BF16, kind="ExternalInput")
    WB_d = nc.dram_tensor("WB", [D, O], BF16, kind="ExternalInput")
    out_d = nc.dram_tensor("out", [CAPM, O], BF16, kind="ExternalOutput")
    # overflow output, transposed+packed: out2[p, c*CAPV + j] is output
    # feature o = c*128 + p of overflow token j
    out2_d = nc.dram_tensor("out2", [P, OCH * CAPV], BF16,
                            kind="ExternalOutput")

    from concourse.tile import TileContext

    with TileContext(nc) as tc:
        with (
            tc.tile_pool(name="const", bufs=1) as const_pool,
            tc.tile_pool(name="xT", bufs=2 * KCH + 2) as xT_pool,
            tc.tile_pool(name="wts", bufs=KCH + 2) as w_pool,
            tc.tile_pool(name="outp", bufs=10) as out_pool,
            tc.tile_pool(name="psum_mm", bufs=8, space="PSUM") as psum_mm,
        ):
            # warm-up operand: one zero tile, memset on the idle Pool
            # engine so the PE dummy chain can start almost immediately
            z = const_pool.tile([P, 256], BF16)
            nc.gpsimd.memset(z, 0.0)

            # input stream, in PE consumption order: group-0 tokens
            # (tiles 0-3) stream as small per-chunk pieces paired with the
            # weight chunks; the group-1 piece, piece B (tiles 8-16) and
            # the merged overflow weight WB follow.
            xTG0 = [None] * KCH
            xTG1 = [None] * KCH
            xTB = [None] * KCH
            WA = [None] * KCH
            xTG0[0] = xT_pool.tile([P, CAPA], BF16, tag="xTG0_0",
                                   name="xTG0_0")
            nc.sync.dma_start(out=xTG0[0], in_=xTA_d[ds(0, P), ds(0, CAPA)])
            WA0h0 = w_pool.tile([P, 512], BF16, tag="WA0h0")
            nc.sync.dma_start(out=WA0h0, in_=WA_d[ds(0, P), ds(0, 512)])
            WA0h1 = w_pool.tile([P, 512], BF16, tag="WA0h1")
            nc.sync.dma_start(out=WA0h1, in_=WA_d[ds(0, P), ds(512, 512)])
            for k in range(1, KCH):
                ta = xT_pool.tile([P, CAPA], BF16, tag="xTG0")
                nc.sync.dma_start(out=ta, in_=xTA_d[ds(k * P, P), ds(0, CAPA)])
                xTG0[k] = ta
                wa = w_pool.tile([P, O], BF16, tag="wa")
                nc.sync.dma_start(out=wa, in_=WA_d[ds(k * P, P), :])
                WA[k] = wa
            for k in range(KCH):
                tg = xT_pool.tile([P, CAPA], BF16, tag="xTG1")
                nc.sync.dma_start(out=tg,
                                  in_=xTA_d[ds(k * P, P), ds(CAPA, CAPA)])
                xTG1[k] = tg
            for k in range(KCH):
                tb = xT_pool.tile([P, CAPB], BF16, tag="xTB")
                nc.sync.dma_start(out=tb, in_=xTB_d[ds(k * P, P), :])
                xTB[k] = tb
            xTV_sb = const_pool.tile([P, KCH, CAPV], BF16)
            nc.sync.dma_start(out=xTV_sb,
                              in_=xTV_d.rearrange("(k p) t -> p k t", p=P))
            WB_sb = const_pool.tile([P, KCH, O], BF16)
            nc.sync.dma_start(out=WB_sb,
                              in_=WB_d.rearrange("(k p) o -> p k o", p=P))

            # PE p-state warm-up: keep the engine busy through the DMA
            # lead-in so real matmuls start at full clock
            psd = psum_mm.tile([P, 512], F32, tag="mm")
            for _ in range(NDUMMY):
                nc.tensor.matmul(psd[:, ds(0, 256)], lhsT=z[:, ds(0, P)],
                                 rhs=z, start=True, stop=True)

            def lhs(k, t):
                if t < TA:
                    return xTG0[k][:, ts(t, P)]
                if t < 2 * TA:
                    return xTG1[k][:, ts(t - TA, P)]
                return xTB[k][:, ts(t - 2 * TA, P)]

            def rhs(k, t, h):
                if k == 0:
                    return (WA0h0 if h == 0 else WA0h1)[:, :]
                return WA[k][:, ds(h * 512, 512)]

            def emit_ovf_group():
                # Overflow tokens computed output-transposed: the weight
                # chunk is stationary (lhsT) and the CAPV=88 token columns
                # stream on the free dim, so the slot costs 88 rows per
                # (o-chunk, k) instead of a padded 128-token tile
                obv = out_pool.tile([P, OCH * CAPV], BF16, tag="obv",
                                    name="obv")
                # two sub-passes of 4 o-chunks so half the PSUM banks are
                # drained and recycled well before the group ends
                for half in range(2):
                    chunks = range(half * OCH // 2, (half + 1) * OCH // 2)
                    psv = {c: psum_mm.tile([P, CAPV], F32, tag="mm",
                                           name=f"psv_{c}") for c in chunks}
                    for k in range(KCH):
                        last = k == KCH - 1
                        for c in chunks:
                            nc.tensor.matmul(psv[c],
                                             lhsT=WB_sb[:, k, ds(c * P, P)],
                                             rhs=xTV_sb[:, k, :],
                                             start=(k == 0), stop=last)
                            if not last:
                                continue
                            if c % 2 == 0:
                                nc.scalar.activation(
                                    obv[:, ds(c * CAPV, CAPV)], psv[c],
                                    mybir.ActivationFunctionType.Copy)
                            else:
                                nc.vector.tensor_copy(
                                    obv[:, ds(c * CAPV, CAPV)], psv[c])
                nc.sync.dma_start(out=out2_d[:, :], in_=obv)

            # k-major groups; drain each PSUM the moment its k=7
            # accumulation lands so banks recycle early
            for tiles in GROUPS:
                if tiles == "OVF":
                    emit_ovf_group()
                    continue
                ps = {(t, h): psum_mm.tile([P, 512], F32, tag="mm",
                                           name=f"ps_{t}_{h}")
                      for t in tiles for h in range(OH)}
                ob = {}
                for k in range(KCH):
                    last = k == KCH - 1
                    if k == 0:
                        order = [(t, h) for h in range(OH) for t in tiles]
                    else:
                        order = [(t, h) for t in tiles for h in range(OH)]
                    for t, h in order:
                        nc.tensor.matmul(ps[t, h], lhsT=lhs(k, t),
                                         rhs=rhs(k, t, h),
                                         start=(k == 0), stop=last)
                        if not last:
                            continue
                        # drain on the two otherwise-idle engines
                        if h == 0:
                            o = out_pool.tile([P, O], BF16, tag="ob",
                                              name=f"ob_{t}")
                            ob[t] = o
                            nc.scalar.activation(
                                o[:, ds(0, 512)], ps[t, h],
                                mybir.ActivationFunctionType.Copy)
                        elif t < TT - 1:
                            nc.vector.tensor_copy(
                                ob[t][:, ds(512, 512)], ps[t, h])
                            nc.sync.dma_start(out=out_d[ts(t, P), :],
                                              in_=ob[t])
                        else:
                            nc.vector.tensor_copy(
                                ob[t][:, ds(512, 256)],
                                ps[t, h][:, ds(0, 256)])
                            nc.scalar.activation(
                                ob[t][:, ds(768, 256)],
                                ps[t, h][:, ds(256, 256)],
                                mybir.ActivationFunctionType.Copy)
                            nc.sync.dma_start(
                                out=out_d[ts(t, P), ds(0, 768)],
                                in_=ob[t][:, ds(0, 768)])
                            nc.sync.dma_start(
                                out=out_d[ts(t, P), ds(768, 256)],
                                in_=ob[t][:, ds(768, 256)])

    nc.compile()
    return nc


_NC_CACHE = None
last_results = None  # BassKernelResults from the most recent run (for test.py)


def _get_nc():
    global _NC_CACHE
    if _NC_CACHE is None:
        _NC_CACHE = _build()
    return _NC_CACHE


def _route(x_flat, Wg, bg):
    """Exact top-2 routing on host (fp64 so selection matches the fp32
    reference even for near-ties; min observed top2-vs-3rd gap is 3e-5)."""
    logits = x_flat.astype(np.float64) @ Wg.astype(np.float64) \
        + bg.astype(np.float64)
    top2 = np.argpartition(-logits, 1, axis=1)[:, :2]          # [N, 2]
    l2 = np.take_along_axis(logits, top2, axis=1)              # [N, 2]
    p = np.exp(l2 - l2.max(axis=1, keepdims=True))
    w2 = (p / p.sum(axis=1, keepdims=True)).astype(np.float32)  # [N, 2]
    return top2, w2


def kernel(x, We, be, Wg, bg):
    global last_results
    x_flat = np.ascontiguousarray(np.asarray(x, np.float32)).reshape(N, D)
    We_np = np.asarray(We, np.float32)
    be_np = np.asarray(be, np.float32)
    top2, w2 = _route(x_flat, np.asarray(Wg, np.float32),
                      np.asarray(bg, np.float32))

    # per-expert token queues (token index + normalized gate weight)
    queues = []
    for e in range(E):
        sel = top2 == e                        # [N, 2] bool
        toks = np.nonzero(sel.any(axis=1))[0]
        wv = w2[toks, sel[toks].argmax(axis=1)]
        queues.append([toks, wv])

    We_bf = We_np.astype(BF16_NP)

    out_acc = np.zeros((N, O), np.float32)
    while any(len(q[0]) for q in queues):
        # greedy largest-remaining-first packing of (expert, token-chunk)
        # into 8 cores x [main slot 2048 | overflow slot 128]
        slots = [[] for _ in range(NCORES)]    # (expert, toks, wv, offset)
        for cap, base in ((CAPM, 0), (CAPV, CAPM)):
            for c in range(NCORES):
                eb = max(range(E), key=lambda e: len(queues[e][0]))
                toks, wv = queues[eb]
                n = min(len(toks), cap)
                if n == 0:
                    continue
                slots[c].append((eb, toks[:n], wv[:n], base))
                queues[eb] = [toks[n:], wv[n:]]

        in_maps = []
        for c in range(NCORES):
            xT_c = np.zeros((D, CAP), np.float32)
            wa = wb = None
            for e, toks, wv, off in slots[c]:
                xT_c[:, off:off + len(toks)] = \
                    (x_flat[toks] * wv[:, None]).T
                if off == 0:
                    wa = We_bf[e]
                else:
                    wb = We_bf[e]
            if wa is None:
                wa = We_bf[0]
            if wb is None:
                wb = wa
            xT_bf = xT_c.astype(BF16_NP)
            in_maps.append(
                {"xTA": np.ascontiguousarray(xT_bf[:, :2 * CAPA]),
                 "xTB": np.ascontiguousarray(xT_bf[:, 2 * CAPA:CAPM]),
                 "xTV": np.ascontiguousarray(xT_bf[:, CAPM:]),
                 "WA": wa, "WB": wb})

        last_results = run_bass_kernel_spmd(_get_nc(), in_maps,
                                            core_ids=list(range(NCORES)))

        # unshard: scatter-add the two scaled expert contributions per
        # token, folding in the gate-weighted bias w*be
        for c in range(NCORES):
            dev = last_results.results[c]["out"]
            dev2 = None
            for e, toks, wv, off in slots[c]:
                n = len(toks)
                if off < CAPM:
                    contrib = dev[off:off + n].astype(np.float32)
                else:
                    if dev2 is None:
                        dev2 = (np.asarray(last_results.results[c]["out2"],
                                           dtype=np.float32)
                                .reshape(P, OCH, CAPV)
                                .transpose(2, 1, 0)
                                .reshape(CAPV, O))
                    contrib = dev2[:n]
                out_acc[toks] += contrib + wv[:, None] * be_np[e][None, :]

    return out_acc.reshape(B, S, O)
